# revision 2
# baseline (speedup 1.0000x reference)
"""CORN ordinal-regression loss kernel v2 for Trainium2 (Bass/Tile), 8-core DP.

Identity: loss[i,k] = softplus(x[i,k]) - bt[i,k]*x[i,k],  bt = (k < rank_i).
Each core handles 524288 rows; host sums 8 partials and divides.

v2 changes vs baseline:
  - bf16 inputs (tolerance is 2e-2; bf16 costs ~1e-4): HBM 21MB -> 10.4MB/core.
  - k-major DRAM layout of logits [9, B_CORE]: each class column is
    contiguous per partition, so the 9 per-column mask STTs on DVE hit the
    2x_1p packed mode (all operands 2-byte, stride 1) instead of 1x strided.
  - ACT strategy selectable:
      "softplus": single Softplus pass (table slot probed on HW),
      "twopass":  Exp then Ln(bias=1) on one table set (baseline scheme),
      "pair":     Exp, q=1+e (DVE 4x), p=q_lo*q_hi (DVE 2x), Ln on half.
"""

import numpy as np
import ml_dtypes

import concourse.bass as bass
import concourse.bacc as bacc_mod
import concourse.tile as tile
from concourse import bacc, mybir
from concourse.bass_utils import run_bass_kernel_spmd
from concourse.tile import add_dep_helper

BATCH = 4_194_304
KM1 = 9
N_CORES = 8
B_CORE = BATCH // N_CORES          # 524288 rows per core
P = 128
T = 512                            # rows per partition per chunk
S = B_CORE // (P * T)              # 8 chunks per core
C = T * KM1
assert S * P * T == B_CORE

MODE = "alt"                       # "softplus" | "twopass" | "pair" | "alt"


def _patch_act_tables():
    """The real gen3 act_info has no set advertising Softplus, but the
    softplus_and_others table binaries exist (act1/act2 custom slots hold
    softplus).  Claim membership so bacc emits LoadActFuncSet+Activate;
    also keep Exp/Ln steered onto their shared set for the fallback modes."""
    if getattr(bacc_mod, "_corn_act_tables_patched_v2", False):
        return
    orig = bacc_mod.get_activation_tables
    AF = mybir.ActivationFunctionType
    both = {AF.Exp, AF.Ln}

    def patched(module_arch):
        tables = dict(orig(module_arch))
        for name, funcs in tables.items():
            s = set(funcs)
            if name == "softplus_and_others":
                s.add(AF.Softplus)
            if name != "natural_log_exp_and_others":
                s -= both
            tables[name] = s
        return tables

    bacc_mod.get_activation_tables = patched
    bacc_mod._corn_act_tables_patched_v2 = True


def build_nc(reps: int = 1, do_compile: bool = True,
             with_act: bool = True, with_masks: bool = True):
    _patch_act_tables()
    nc = bacc.Bacc("TRN2", target_bir_lowering=False, debug=False,
                   num_devices=N_CORES)
    f32 = mybir.dt.float32
    bf16 = mybir.dt.bfloat16
    AF = mybir.ActivationFunctionType
    OP = mybir.AluOpType

    x_d = nc.dram_tensor("logits", [KM1, B_CORE], bf16, kind="ExternalInput")
    t_d = nc.dram_tensor("tr", [B_CORE], bf16, kind="ExternalInput")
    o_d = nc.dram_tensor("partial", [1, 1], f32, kind="ExternalOutput")

    xv = x_d.ap().rearrange("k (s p t) -> s p k t", p=P, t=T)  # [S,128,9,T]
    tv = t_d.ap().rearrange("(s p t) -> s p t", p=P, t=T)      # [S,128,T]

    with tile.TileContext(nc) as tc:
        with (
            tc.tile_pool(name="xin", bufs=3) as xpool,
            tc.tile_pool(name="tin", bufs=3) as tpool,
            tc.tile_pool(name="work", bufs=4) as wpool,
            tc.tile_pool(name="dummy", bufs=1) as dpool,
            tc.tile_pool(name="acc", bufs=1) as apool,
            tc.tile_pool(name="psum", bufs=1, space="PSUM") as ppool,
        ):
            chunks = [(s, 0, T) for s in range(S)]
            n_chunks = len(chunks)
            sp_acc = apool.tile([P, n_chunks], f32)
            bx_acc = apool.tile([P, n_chunks * KM1], f32)
            l_dump = dpool.tile([P, C], bf16)
            s_dump = dpool.tile([P, T], bf16)

            def body(_i=None):
                exps, lns = [], []
                if not with_act:
                    nc.vector.memset(sp_acc[:], 0.0)
                if not with_masks:
                    nc.vector.memset(bx_acc[:], 0.0)
                for ci, (s, t0, tn) in enumerate(chunks):
                    cn, hn = tn * KM1, tn * KM1 // 2
                    tr_t = tpool.tile([P, tn], bf16, tag=f"tr{tn}")
                    nc.sync.dma_start(out=tr_t[:], in_=tv[s][:, t0:t0 + tn])
                    x_t = xpool.tile([P, KM1, tn], bf16, tag=f"x{tn}")
                    nc.sync.dma_start(out=x_t[:], in_=xv[s][:, :, t0:t0 + tn])
                    xf = x_t.rearrange("p k t -> p (k t)")

                    # masks first in DVE program order: they depend only on
                    # the DMAs, so they never wait on ACT the way q does.
                    for k in range(KM1 if with_masks else 0):
                        nc.vector.scalar_tensor_tensor(
                            out=s_dump[:, :tn], in0=tr_t[:], scalar=float(k),
                            in1=x_t[:, k, :],
                            op0=OP.is_gt, op1=OP.mult,
                            accum_out=bx_acc[:, ci * KM1 + k:ci * KM1 + k + 1],
                        )

                    mode = MODE
                    if MODE == "alt":
                        # pair chunks halve ACT's Ln work but add q/p on DVE;
                        # alternating balances the two engine streams.
                        mode = "pair" if ci % 2 == 0 else "twopass"
                    if not with_act:
                        pass
                    elif mode == "softplus":
                        lns.append(nc.scalar.activation(
                            l_dump[:, :cn], xf[:], AF.Softplus,
                            accum_out=sp_acc[:, ci:ci + 1]))
                    elif mode == "twopass":
                        e_t = wpool.tile([P, cn], bf16, tag=f"exp{tn}")
                        exps.append(nc.scalar.activation(e_t[:], xf[:], AF.Exp))
                        lns.append(nc.scalar.activation(
                            l_dump[:, :cn], e_t[:], AF.Ln, bias=1.0,
                            accum_out=sp_acc[:, ci:ci + 1]))
                    else:  # pair
                        e_t = wpool.tile([P, cn], bf16, tag=f"exp{tn}")
                        exps.append(nc.scalar.activation(e_t[:], xf[:], AF.Exp))
                        q_t = wpool.tile([P, cn], bf16, tag=f"q{tn}")
                        nc.vector.tensor_scalar(
                            out=q_t[:], in0=e_t[:], scalar1=1.0,
                            scalar2=None, op0=OP.add)
                        p_t = wpool.tile([P, hn], bf16, tag=f"p{tn}")
                        nc.vector.tensor_tensor(
                            p_t[:], q_t[:, :hn], q_t[:, hn:], OP.mult)
                        lns.append(nc.scalar.activation(
                            l_dump[:, :hn], p_t[:], AF.Ln,
                            accum_out=sp_acc[:, ci:ci + 1]))

                if with_act and MODE in ("twopass", "pair", "alt"):
                    for ci in range(n_chunks - 1):
                        add_dep_helper(lns[ci].ins, exps[ci + 1].ins,
                                       sync=False,
                                       reason="keep exp ahead of ln on ACT")

            if reps == 1:
                body()
            else:
                with tc.For_i(0, reps, 1) as i:
                    body(i)

            r_sp = apool.tile([P, 1], f32)
            nc.vector.tensor_reduce(r_sp[:], sp_acc[:],
                                    axis=mybir.AxisListType.X, op=OP.add)
            r_bx = apool.tile([P, 1], f32)
            nc.vector.tensor_reduce(r_bx[:], bx_acc[:],
                                    axis=mybir.AxisListType.X, op=OP.add)
            diff = apool.tile([P, 1], f32)
            nc.vector.tensor_tensor(diff[:], r_sp[:], r_bx[:], OP.subtract)
            ones = apool.tile([P, 1], f32)
            nc.vector.memset(ones[:], 1.0)
            ps = ppool.tile([1, 1], f32)
            nc.tensor.matmul(out=ps[:], lhsT=ones[:], rhs=diff[:],
                             start=True, stop=True)
            res = apool.tile([1, 1], f32)
            nc.vector.tensor_copy(out=res[:], in_=ps[:])
            nc.sync.dma_start(out=o_d.ap(), in_=res[:])
    if do_compile:
        nc.compile()
    return nc


_NC_CACHE: dict[int, object] = {}


def _get_nc(reps: int = 1):
    if reps not in _NC_CACHE:
        _NC_CACHE[reps] = build_nc(reps)
    return _NC_CACHE[reps]


def make_in_maps(logits: np.ndarray, targets: np.ndarray):
    tr = (np.asarray(targets).astype(np.float32) - 1.0).astype(
        ml_dtypes.bfloat16)
    lg = np.asarray(logits).astype(ml_dtypes.bfloat16)
    return [
        {
            "logits": np.ascontiguousarray(
                lg[c * B_CORE:(c + 1) * B_CORE].T),
            "tr": tr[c * B_CORE:(c + 1) * B_CORE],
        }
        for c in range(N_CORES)
    ]


def kernel(logits: np.ndarray, targets: np.ndarray) -> np.ndarray:
    nc = _get_nc(1)
    in_maps = make_in_maps(logits, targets)
    r = run_bass_kernel_spmd(nc, in_maps, core_ids=list(range(N_CORES)))
    total = sum(float(res["partial"][0, 0]) for res in r.results)
    return np.float32(total / (BATCH * KM1))


if __name__ == "__main__":
    rng = np.random.default_rng(0)
    lg = rng.standard_normal((BATCH, KM1)).astype(np.float32)
    tg = rng.integers(1, 11, size=(BATCH,)).astype(np.int64)
    out = kernel(lg, tg)
    ks = np.arange(KM1)
    bt = (ks[None, :] < (tg - 1)[:, None]).astype(np.float64)
    lgb = lg.astype(ml_dtypes.bfloat16).astype(np.float64)
    sp = np.log1p(np.exp(lgb))
    want = (sp - bt * lgb).mean()
    print("kernel:", out, "ref:", want, "relerr:", abs(out - want) / abs(want))


# revision 7
# speedup vs baseline: 1.0166x; 1.0166x over previous
"""CORN ordinal-regression loss kernel for Trainium2 (Bass/Tile), 8-core DP.

Identity: loss[i,k] = softplus(x[i,k]) - bt[i,k]*x[i,k],  bt = (k < rank_i).
Each core handles 524288 rows; host sums 8 partials and divides.
Measured 67-68 us/pass on trn2 (steady-state repeat-loop), rel err 3.2e-5
vs the f32 reference; previous Exp+Ln/f32 version measured ~100 us.

Design notes (engine budgets per core, from the cost model and verified
against HW within 3%):
  - bf16 inputs (tolerance is 2e-2; bf16 costs ~3e-5 here): HBM traffic
    21MB -> 10.4MB/core, DMA ~31us at ~330GB/s effective.
  - k-major DRAM layout of logits [9, B_CORE]: each class column is
    contiguous per partition (stride-1 bf16 operands for the mask STTs).
  - The 9 per-column mask STTs are DVE 1x (scalar_tensor_tensor has no
    2x uop; measured and cost-model-confirmed): ~43us total, and the fused
    compare*mult+accum form is still cheaper than any stock-op alternative
    (tensor_scalar is 4x and tensor_tensor 2x, but neither reduces; the
    3-op decompositions all exceed 1 fused pass).
  - ACT has no Softplus table on gen3 (probed: the softplus_and_others
    act1/act2 slots do NOT answer AF.Softplus), so softplus costs two
    table passes (Exp, Ln(bias=1), one shared table set).  MODE="alt"
    gives half the chunks the pair-product treatment (q=1+e on DVE 4x,
    p=q_lo*q_hi on DVE 2x, Ln on half the elements), balancing
    ACT ~54us vs DVE ~56us; f=1 or f=0 are both slower (80/76us sim).
"""

import numpy as np
import ml_dtypes

import concourse.bass as bass
import concourse.bacc as bacc_mod
import concourse.tile as tile
from concourse import bacc, mybir
from concourse.bass_utils import run_bass_kernel_spmd
from concourse.tile import add_dep_helper

BATCH = 4_194_304
KM1 = 9
N_CORES = 8
B_CORE = BATCH // N_CORES          # 524288 rows per core
P = 128
T = 512                            # rows per partition per chunk
S = B_CORE // (P * T)              # 8 chunks per core
C = T * KM1
assert S * P * T == B_CORE

MODE = "alt"                       # "softplus" | "twopass" | "pair" | "alt"
# Ramp/tail chunk splitting was measured SLOWER in sim (73.3us at 4/2 vs
# 69.4us at 1/1): the extra per-op inits on ACT/DVE outweigh the shorter
# first-DMA ramp, and small chunks push work back to the twopass path.
# Keep 1/1.  Offloading the pair product to GPSIMD/Pool and raising the
# pair fraction also simmed slower (72-74us) - the three streams are
# rate-matched (DMA ~33us pace, ACT ~54us, DVE ~56us busy) and the wall
# is their coupling, not any single engine.
START_DIV = 1
TAIL_DIV = 1


def _patch_act_tables():
    """The real gen3 act_info has no set advertising Softplus, but the
    softplus_and_others table binaries exist (act1/act2 custom slots hold
    softplus).  Claim membership so bacc emits LoadActFuncSet+Activate;
    also keep Exp/Ln steered onto their shared set for the fallback modes."""
    if getattr(bacc_mod, "_corn_act_tables_patched_v2", False):
        return
    orig = bacc_mod.get_activation_tables
    AF = mybir.ActivationFunctionType
    both = {AF.Exp, AF.Ln}

    def patched(module_arch):
        tables = dict(orig(module_arch))
        for name, funcs in tables.items():
            s = set(funcs)
            if name == "softplus_and_others":
                s.add(AF.Softplus)
            if name != "natural_log_exp_and_others":
                s -= both
            tables[name] = s
        return tables

    bacc_mod.get_activation_tables = patched
    bacc_mod._corn_act_tables_patched_v2 = True


def build_nc(reps: int = 1, do_compile: bool = True,
             with_act: bool = True, with_masks: bool = True):
    _patch_act_tables()
    nc = bacc.Bacc("TRN2", target_bir_lowering=False, debug=False,
                   num_devices=N_CORES)
    f32 = mybir.dt.float32
    bf16 = mybir.dt.bfloat16
    AF = mybir.ActivationFunctionType
    OP = mybir.AluOpType

    x_d = nc.dram_tensor("logits", [KM1, B_CORE], bf16, kind="ExternalInput")
    t_d = nc.dram_tensor("tr", [B_CORE], bf16, kind="ExternalInput")
    o_d = nc.dram_tensor("partial", [1, 1], f32, kind="ExternalOutput")

    xv = x_d.ap().rearrange("k (s p t) -> s p k t", p=P, t=T)  # [S,128,9,T]
    tv = t_d.ap().rearrange("(s p t) -> s p t", p=P, t=T)      # [S,128,T]

    with tile.TileContext(nc) as tc:
        with (
            tc.tile_pool(name="xin", bufs=3) as xpool,
            tc.tile_pool(name="tin", bufs=3) as tpool,
            tc.tile_pool(name="work", bufs=4) as wpool,
            tc.tile_pool(name="dummy", bufs=1) as dpool,
            tc.tile_pool(name="acc", bufs=1) as apool,
            tc.tile_pool(name="psum", bufs=1, space="PSUM") as ppool,
        ):
            chunks = [(0, i * (T // START_DIV), T // START_DIV)
                      for i in range(START_DIV)]
            chunks += [(s, 0, T) for s in range(1, S - 1)]
            chunks += [(S - 1, i * (T // TAIL_DIV), T // TAIL_DIV)
                       for i in range(TAIL_DIV)]
            n_chunks = len(chunks)
            n_full = sum(1 for (_, _, tn) in chunks if tn == T)
            sp_acc = apool.tile([P, n_chunks], f32)
            bx_acc = apool.tile([P, n_chunks * KM1], f32)
            l_dump = dpool.tile([P, C], bf16)
            s_dump = dpool.tile([P, T], bf16)

            def body(_i=None):
                exps, lns = [], []
                if not with_act:
                    nc.vector.memset(sp_acc[:], 0.0)
                if not with_masks:
                    nc.vector.memset(bx_acc[:], 0.0)
                for ci, (s, t0, tn) in enumerate(chunks):
                    cn, hn = tn * KM1, tn * KM1 // 2
                    tr_t = tpool.tile([P, tn], bf16, tag=f"tr{tn}")
                    nc.sync.dma_start(out=tr_t[:], in_=tv[s][:, t0:t0 + tn])
                    x_t = xpool.tile([P, KM1, tn], bf16, tag=f"x{tn}")
                    nc.sync.dma_start(out=x_t[:], in_=xv[s][:, :, t0:t0 + tn])
                    xf = x_t.rearrange("p k t -> p (k t)")

                    # masks first in DVE program order: they depend only on
                    # the DMAs, so they never wait on ACT the way q does.
                    for k in range(KM1 if with_masks else 0):
                        nc.vector.scalar_tensor_tensor(
                            out=s_dump[:, :tn], in0=tr_t[:], scalar=float(k),
                            in1=x_t[:, k, :],
                            op0=OP.is_gt, op1=OP.mult,
                            accum_out=bx_acc[:, ci * KM1 + k:ci * KM1 + k + 1],
                        )

                    mode = MODE
                    if MODE == "alt":
                        # pair chunks halve ACT's Ln work but add q/p on DVE;
                        # alternating full chunks balances the two engine
                        # streams.  Small (ramp/tail) chunks use the short
                        # exp->ln chain so the drain after the last DMA is
                        # as short as possible.
                        if tn < T:
                            mode = "twopass"
                        else:
                            mode = "pair" if ci % 2 == 0 else "twopass"
                    if not with_act:
                        pass
                    elif mode == "softplus":
                        lns.append(nc.scalar.activation(
                            l_dump[:, :cn], xf[:], AF.Softplus,
                            accum_out=sp_acc[:, ci:ci + 1]))
                    elif mode == "twopass":
                        e_t = wpool.tile([P, cn], bf16, tag=f"exp{tn}")
                        exps.append(nc.scalar.activation(e_t[:], xf[:], AF.Exp))
                        lns.append(nc.scalar.activation(
                            l_dump[:, :cn], e_t[:], AF.Ln, bias=1.0,
                            accum_out=sp_acc[:, ci:ci + 1]))
                    else:  # pair
                        e_t = wpool.tile([P, cn], bf16, tag=f"exp{tn}")
                        exps.append(nc.scalar.activation(e_t[:], xf[:], AF.Exp))
                        q_t = wpool.tile([P, cn], bf16, tag=f"q{tn}")
                        nc.vector.tensor_scalar(
                            out=q_t[:], in0=e_t[:], scalar1=1.0,
                            scalar2=None, op0=OP.add)
                        p_t = wpool.tile([P, hn], bf16, tag=f"p{tn}")
                        nc.vector.tensor_tensor(
                            p_t[:], q_t[:, :hn], q_t[:, hn:], OP.mult)
                        lns.append(nc.scalar.activation(
                            l_dump[:, :hn], p_t[:], AF.Ln,
                            accum_out=sp_acc[:, ci:ci + 1]))

                if with_act and MODE in ("twopass", "pair", "alt"):
                    for ci in range(n_chunks - 1):
                        add_dep_helper(lns[ci].ins, exps[ci + 1].ins,
                                       sync=False,
                                       reason="keep exp ahead of ln on ACT")

            if reps == 1:
                body()
            else:
                with tc.For_i(0, reps, 1) as i:
                    body(i)

            r_sp = apool.tile([P, 1], f32)
            nc.vector.tensor_reduce(r_sp[:], sp_acc[:],
                                    axis=mybir.AxisListType.X, op=OP.add)
            r_bx = apool.tile([P, 1], f32)
            nc.vector.tensor_reduce(r_bx[:], bx_acc[:],
                                    axis=mybir.AxisListType.X, op=OP.add)
            diff = apool.tile([P, 1], f32)
            nc.vector.tensor_tensor(diff[:], r_sp[:], r_bx[:], OP.subtract)
            ones = apool.tile([P, 1], f32)
            nc.vector.memset(ones[:], 1.0)
            ps = ppool.tile([1, 1], f32)
            nc.tensor.matmul(out=ps[:], lhsT=ones[:], rhs=diff[:],
                             start=True, stop=True)
            res = apool.tile([1, 1], f32)
            nc.vector.tensor_copy(out=res[:], in_=ps[:])
            nc.sync.dma_start(out=o_d.ap(), in_=res[:])
    if do_compile:
        nc.compile()
    return nc


_NC_CACHE: dict[int, object] = {}


def _get_nc(reps: int = 1):
    if reps not in _NC_CACHE:
        _NC_CACHE[reps] = build_nc(reps)
    return _NC_CACHE[reps]


def make_in_maps(logits: np.ndarray, targets: np.ndarray):
    tr = (np.asarray(targets).astype(np.float32) - 1.0).astype(
        ml_dtypes.bfloat16)
    lg = np.asarray(logits).astype(ml_dtypes.bfloat16)
    return [
        {
            "logits": np.ascontiguousarray(
                lg[c * B_CORE:(c + 1) * B_CORE].T),
            "tr": tr[c * B_CORE:(c + 1) * B_CORE],
        }
        for c in range(N_CORES)
    ]


def kernel(logits: np.ndarray, targets: np.ndarray) -> np.ndarray:
    nc = _get_nc(1)
    in_maps = make_in_maps(logits, targets)
    r = run_bass_kernel_spmd(nc, in_maps, core_ids=list(range(N_CORES)))
    total = sum(float(res["partial"][0, 0]) for res in r.results)
    return np.float32(total / (BATCH * KM1))


if __name__ == "__main__":
    rng = np.random.default_rng(0)
    lg = rng.standard_normal((BATCH, KM1)).astype(np.float32)
    tg = rng.integers(1, 11, size=(BATCH,)).astype(np.int64)
    out = kernel(lg, tg)
    ks = np.arange(KM1)
    bt = (ks[None, :] < (tg - 1)[:, None]).astype(np.float64)
    lgb = lg.astype(ml_dtypes.bfloat16).astype(np.float64)
    sp = np.log1p(np.exp(lgb))
    want = (sp - bt * lgb).mean()
    print("kernel:", out, "ref:", want, "relerr:", abs(out - want) / abs(want))
